# revision 23
# baseline (speedup 1.0000x reference)
"""Trainium2 Bass kernel for attention-pooling module.

reference:
  pos_proj = position @ W1 + b1                  [B, 1, U]
  opt_proj = options @ W2 + b2                   [B, S, U]
  score    = tanh(pos_proj + opt_proj) @ V + bv  [B, S, 1]
  weights  = softmax(score, axis=1)
  context  = sum(weights * options, axis=1)      [B, D]
  returns (context, weights)

B=32, S=4096, D=256, U=256. Data-parallel over batch: 4 batches/core on 8
cores. The host pre-casts options to bf16 in BOTH [S,D] and [D,S] layouts
(same 16 MiB/core of DMA as one f32 copy), so the device does zero
transposes:
  pass 1: projection matmul from the [D,S] layout (bf16)
          -> fused tanh+bias on ScalarE -> score matmul against V
  softmax: no max-subtraction (scores are O(1); exact shift-invariance also
           lets us drop bv entirely); score rows are gathered across
           partitions with tiny SBUF->SBUF DMAs, then 4 PE transposes
  pass 2: context = weights^T @ options from the [S,D] layout (bf16).
"""

import numpy as np
from contextlib import ExitStack

import concourse.bass as bass
import concourse.tile as tile
from concourse import bacc
import concourse.mybir as mybir

F32 = mybir.dt.float32
BF16 = mybir.dt.bfloat16
AF = mybir.ActivationFunctionType

B, S, D, U = 32, 4096, 256, 256
NCORES = 8
BL = B // NCORES        # 4 local batches per core
NT5 = S // 512          # 8 tiles of 512 sequence positions
NT1 = S // 128          # 32 tiles of 128
NCOL = BL * NT1         # 128 score columns, c = b*32 + j*8 + t5


def build_nc():
    nc = bacc.Bacc(None, target_bir_lowering=False)
    pos_e = nc.declare_dram_parameter("position", [BL, D], F32, isOutput=False)
    opt_e = nc.declare_dram_parameter("options_bf", [BL, S, D], BF16,
                                      isOutput=False)
    optT_e = nc.declare_dram_parameter("optionsT_bf", [BL, D, S], BF16,
                                       isOutput=False)
    w1_e = nc.declare_dram_parameter("W1", [D, U], F32, isOutput=False)
    b1_e = nc.declare_dram_parameter("b1", [U], F32, isOutput=False)
    w2_e = nc.declare_dram_parameter("W2", [D, U], F32, isOutput=False)
    b2_e = nc.declare_dram_parameter("b2", [U], F32, isOutput=False)
    v_e = nc.declare_dram_parameter("V", [U, 1], F32, isOutput=False)
    ctx_e = nc.declare_dram_parameter("context", [BL, D], F32, isOutput=True)
    wts_e = nc.declare_dram_parameter("weights", [BL, S, 1], F32, isOutput=True)

    with tile.TileContext(nc) as tc, ExitStack() as ctx:
        const = ctx.enter_context(tc.tile_pool(name="const", bufs=1))
        optT_pool = ctx.enter_context(tc.tile_pool(name="optT", bufs=8))
        tanh_pool = ctx.enter_context(tc.tile_pool(name="tanh", bufs=8))
        srow_sb_pool = ctx.enter_context(tc.tile_pool(name="srowsb", bufs=4))

        # ---- constants -----------------------------------------------------
        ones = const.tile([128, 128], F32)
        nc.vector.memset(ones[:], 1.0)
        ident = const.tile([128, 128], F32)
        nc.gpsimd.affine_select(
            ident[:], ones[:], pattern=[[1, 128]],
            compare_op=mybir.AluOpType.is_equal, fill=0.0,
            base=0, channel_multiplier=-1,
        )
        # bm128T[b, c] = 1 when score column c belongs to batch b (c//32 == b)
        bm_lo = const.tile([BL, NCOL], F32)
        nc.gpsimd.affine_select(
            bm_lo[:], ones[0:BL, :], pattern=[[1, NCOL]],
            compare_op=mybir.AluOpType.is_ge, fill=0.0,
            base=0, channel_multiplier=-NT1,
        )
        bm_hi = const.tile([BL, NCOL], F32)
        nc.gpsimd.affine_select(
            bm_hi[:], ones[0:BL, :], pattern=[[1, NCOL]],
            compare_op=mybir.AluOpType.is_ge, fill=0.0,
            base=-NT1, channel_multiplier=-NT1,
        )
        bm128T = const.tile([BL, NCOL], F32)
        nc.vector.tensor_tensor(bm128T[:], bm_lo[:], bm_hi[:],
                                mybir.AluOpType.subtract)

        # ---- weights -------------------------------------------------------
        w2f = const.tile([128, 2, U], F32)
        nc.scalar.dma_start(w2f[:], w2_e[:].rearrange("(c p) u -> p c u", p=128))
        w2b = const.tile([128, 2, U], BF16)
        nc.vector.tensor_copy(w2b[:], w2f[:])
        w1f = const.tile([128, 2, U], F32)
        nc.scalar.dma_start(w1f[:], w1_e[:].rearrange("(c p) u -> p c u", p=128))
        vf = const.tile([128, 2, 1], F32)
        nc.scalar.dma_start(vf[:], v_e[:].rearrange("(c p) o -> p c o", p=128))
        vb = const.tile([128, 2, 1], BF16)
        nc.vector.tensor_copy(vb[:], vf[:])

        b1f = const.tile([1, U], F32)
        nc.scalar.dma_start(b1f[:], b1_e[:].rearrange("(o u) -> o u", o=1))
        b2f = const.tile([1, U], F32)
        nc.scalar.dma_start(b2f[:], b2_e[:].rearrange("(o u) -> o u", o=1))
        b12 = const.tile([1, U], F32)
        nc.vector.tensor_tensor(b12[:], b1f[:], b2f[:], mybir.AluOpType.add)

        posf = const.tile([BL, D], F32)
        nc.scalar.dma_start(posf[:], pos_e[:])

        # ---- options: natural bf16 resident (for pass 2) --------------------
        optb = {}
        for b in range(BL):
            ob = const.tile([128, NT5, 4, D], BF16, tag=f"optb_{b}")
            nc.gpsimd.dma_start(
                ob[:],
                opt_e[b].rearrange("(f t p) d -> p f t d", p=128, t=4))
            optb[b] = ob

        # packed score rows [r=(b,t5), s_local]
        packed = const.tile([BL * NT5, 512], F32)
        # scoreT columns [s_lo, (b, j, t5)]
        scoreT = const.tile([128, BL, 4, NT5], F32)

        # prologue: transpose b1+b2 and position into U-on-partitions layout
        with ExitStack() as prol_scope:
            prol = prol_scope.enter_context(
                tc.tile_pool(name="prol", bufs=1, space="PSUM"))
            b12T = const.tile([128, 2], F32)
            posT = const.tile([128, 2, BL], F32)
            for c in range(2):
                tp = prol.tile([128, 128], F32, tag="smalltr")
                nc.tensor.transpose(tp[0:128, 0:1], b12[:, c * 128:(c + 1) * 128],
                                    ident[0:1, 0:1])
                nc.vector.tensor_copy(b12T[:, c:c + 1], tp[0:128, 0:1])
                tp2 = prol.tile([128, 128], F32, tag="smalltr")
                nc.tensor.transpose(tp2[0:128, 0:BL],
                                    posf[:, c * 128:(c + 1) * 128],
                                    ident[0:BL, 0:BL])
                nc.vector.tensor_copy(posT[:, c, :], tp2[0:128, 0:BL])

            # posbias[u, uc, b] = (position @ W1)[b, u] + b1[u] + b2[u]
            posbias = const.tile([128, 2, BL], F32)
            for uc in range(2):
                pb = prol.tile([128, 128], F32, tag="smalltr")
                for dc in range(2):
                    nc.tensor.matmul(pb[0:128, 0:BL],
                                     w1f[:, dc, uc * 128:(uc + 1) * 128],
                                     posT[:, dc, :],
                                     start=(dc == 0), stop=(dc == 1))
                nc.scalar.activation(posbias[:, uc, :], pb[0:128, 0:BL],
                                     AF.Identity, bias=b12T[:, uc:uc + 1])

        with ExitStack() as p1:
            proj_pool = p1.enter_context(
                tc.tile_pool(name="proj", bufs=4, space="PSUM"))
            srow_pool = p1.enter_context(
                tc.tile_pool(name="srow", bufs=4, space="PSUM"))

            # ---- pass 1: scores -------------------------------------------
            for b in range(BL):
                for t5 in range(NT5):
                    r = b * NT5 + t5
                    oT = optT_pool.tile([128, 2, 512], BF16)
                    nc.sync.dma_start(
                        oT[:],
                        optT_e[b, :, t5 * 512:(t5 + 1) * 512].rearrange(
                            "(c p) s -> p c s", p=128),
                    )
                    srow = srow_pool.tile([1, 512], F32)
                    for uc in range(2):
                        pj = proj_pool.tile([128, 512], F32)
                        for dc in range(2):
                            nc.tensor.matmul(
                                pj[:], w2b[:, dc, uc * 128:(uc + 1) * 128],
                                oT[:, dc, :],
                                start=(dc == 0), stop=(dc == 1))
                        th = tanh_pool.tile([128, 512], BF16)
                        nc.scalar.activation(th[:], pj[:], AF.Tanh,
                                             bias=posbias[:, uc, b:b + 1])
                        nc.tensor.matmul(srow[:], vb[:, uc, :], th[:],
                                         start=(uc == 0), stop=(uc == 1))
                    srow_sb = srow_sb_pool.tile([1, 512], F32)
                    nc.vector.tensor_copy(srow_sb[:], srow[:])
                    # gather this row across partitions via a tiny DMA
                    nc.gpsimd.dma_start(packed[r:r + 1, :], srow_sb[:])

        # ---- softmax + pass 2 ---------------------------------------------
        with ExitStack() as p2:
            strx = p2.enter_context(tc.tile_pool(name="strx", bufs=2,
                                                 space="PSUM"))
            eps = p2.enter_context(tc.tile_pool(name="eps", bufs=1,
                                                space="PSUM"))

            # transpose packed rows into scoreT columns (c = b*32 + j*8 + t5)
            for j in range(4):
                tp = strx.tile([128, 32], F32)
                nc.tensor.transpose(tp[:], packed[:, j * 128:(j + 1) * 128],
                                    ident[0:BL * NT5, 0:BL * NT5])
                nc.vector.tensor_copy(
                    scoreT[:, :, j, :],
                    tp[:].rearrange("p (b t) -> p b t", b=BL))

            expT = const.tile([128, NCOL], F32)
            score_flat = scoreT[:].rearrange("p b j t -> p (b j t)")
            nc.scalar.activation(expT[:], score_flat, AF.Exp)

            colsum_ps = eps.tile([1, NCOL], F32, tag="colsum")
            nc.tensor.matmul(colsum_ps[:], ones[:, 0:1], expT[:])
            colsum = const.tile([1, NCOL], F32)
            nc.vector.tensor_copy(colsum[:], colsum_ps[:])
            zrow = const.tile([1, BL], F32)
            nc.vector.tensor_reduce(
                zrow[:], colsum[:].rearrange("o (b t) -> o b t", b=BL),
                mybir.AxisListType.X, mybir.AluOpType.add)
            invzr = const.tile([1, BL], F32)
            nc.vector.reciprocal(invzr[:], zrow[:])
            invz_ps = eps.tile([BL, 1], F32, tag="invz")
            nc.tensor.transpose(invz_ps[:], invzr[:], ident[0:1, 0:1])
            invz4 = const.tile([BL, 1], F32)
            nc.vector.tensor_copy(invz4[:], invz_ps[:])

            # invZmat[p, c] = 1/Z_{batch(c)}  via ones^T @ (bm128T * invz4)
            bmsc = const.tile([BL, NCOL], F32)
            nc.vector.tensor_scalar(bmsc[:], bm128T[:], invz4[:], None,
                                    mybir.AluOpType.mult)
            invzmat_ps = eps.tile([128, NCOL], F32, tag="invzmat")
            nc.tensor.matmul(invzmat_ps[:], ones[0:BL, :], bmsc[:])

            # normalized weights, transposed layout [s_lo, c]
            wT = const.tile([128, NCOL], F32)
            nc.vector.tensor_tensor(wT[:], expT[:], invzmat_ps[:],
                                    mybir.AluOpType.mult)
            wTb = const.tile([128, NCOL], BF16)
            nc.vector.tensor_copy(wTb[:], wT[:])

            # back to row layout [(b,j,t5), s_lo] for a contiguous DMA out
            wrows_ps = eps.tile([128, 128], F32, tag="wrows")
            nc.tensor.transpose(wrows_ps[:], wT[:], ident[:])
            wrows = const.tile([128, 128], F32)
            nc.vector.tensor_copy(wrows[:], wrows_ps[:])
            for b in range(BL):
                wv = wts_e[b].rearrange("(t j p) o -> j t (p o)", p=128, j=4)
                for j in range(4):
                    nc.gpsimd.dma_start(
                        wv[j],
                        wrows[b * NT1 + j * NT5:b * NT1 + (j + 1) * NT5, :])

            # pass 2: context[b] = sum_s w[b,s] * options[b,s,:]  (bf16)
            ctx_ps = eps.tile([1, BL * D], F32, tag="ctxps")
            for b in range(BL):
                k = 0
                for t5 in range(NT5):
                    for j in range(4):
                        c = b * NT1 + j * NT5 + t5
                        nc.tensor.matmul(
                            ctx_ps[0:1, b * D:(b + 1) * D],
                            wTb[:, c:c + 1],
                            optb[b][:, t5, j, :],
                            start=(k == 0), stop=(k == NT1 - 1))
                        k += 1
            ctxrow = const.tile([1, BL * D], F32)
            nc.vector.tensor_copy(ctxrow[:], ctx_ps[:])
            nc.sync.dma_start(
                ctx_e[:].rearrange("(o b) d -> o (b d)", o=1), ctxrow[:])
    nc.finalize()
    return nc


_NC = None


def _get_nc():
    global _NC
    if _NC is None:
        _NC = build_nc()
    return _NC


def make_in_maps(position, options, W1, b1, W2, b2, V, bv=None):
    import ml_dtypes
    position = np.ascontiguousarray(position, dtype=np.float32)
    options = np.asarray(options, dtype=np.float32)
    opt_bf = np.ascontiguousarray(options.astype(ml_dtypes.bfloat16))
    optT_bf = np.ascontiguousarray(opt_bf.transpose(0, 2, 1))
    W1 = np.ascontiguousarray(W1, dtype=np.float32)
    b1 = np.ascontiguousarray(b1, dtype=np.float32)
    W2 = np.ascontiguousarray(W2, dtype=np.float32)
    b2 = np.ascontiguousarray(b2, dtype=np.float32)
    V = np.ascontiguousarray(V, dtype=np.float32)
    in_maps = []
    for c in range(NCORES):
        bs = slice(c * BL, (c + 1) * BL)
        in_maps.append({
            "position": position[bs], "options_bf": opt_bf[bs],
            "optionsT_bf": optT_bf[bs],
            "W1": W1, "b1": b1, "W2": W2, "b2": b2, "V": V,
        })
    return in_maps


def run_spmd(in_maps, trace=False):
    from concourse.bass_utils import run_bass_kernel_spmd
    return run_bass_kernel_spmd(_get_nc(), in_maps, list(range(NCORES)),
                                trace=trace)


def kernel(position, options, W1, b1, W2, b2, V, bv):
    res = run_spmd(make_in_maps(position, options, W1, b1, W2, b2, V)).results
    context = np.concatenate([res[c]["context"] for c in range(NCORES)], axis=0)
    weights = np.concatenate([res[c]["weights"] for c in range(NCORES)], axis=0)
    return context, weights


# revision 25
# speedup vs baseline: 1.0099x; 1.0099x over previous
"""Trainium2 Bass kernel for attention-pooling module.

reference:
  pos_proj = position @ W1 + b1                  [B, 1, U]
  opt_proj = options @ W2 + b2                   [B, S, U]
  score    = tanh(pos_proj + opt_proj) @ V + bv  [B, S, 1]
  weights  = softmax(score, axis=1)
  context  = sum(weights * options, axis=1)      [B, D]
  returns (context, weights)

B=32, S=4096, D=256, U=256. Data-parallel over batch: 4 batches/core on 8
cores. The host pre-casts options to bf16 in BOTH [S,D] and [D,S] layouts
(same 16 MiB/core of DMA as one f32 copy), so the device does zero
transposes:
  pass 1: projection matmul from the [D,S] layout (bf16)
          -> fused tanh+bias on ScalarE -> score matmul against V
  softmax: no max-subtraction (scores are O(1); exact shift-invariance also
           lets us drop bv entirely); score rows are gathered across
           partitions with tiny SBUF->SBUF DMAs, then 4 PE transposes
  pass 2: context = weights^T @ options from the [S,D] layout (bf16).
"""

import numpy as np
from contextlib import ExitStack

import concourse.bass as bass
import concourse.tile as tile
from concourse import bacc
import concourse.mybir as mybir

F32 = mybir.dt.float32
BF16 = mybir.dt.bfloat16
AF = mybir.ActivationFunctionType

B, S, D, U = 32, 4096, 256, 256
NCORES = 8
BL = B // NCORES        # 4 local batches per core
NT5 = S // 512          # 8 tiles of 512 sequence positions
NT1 = S // 128          # 32 tiles of 128
NCOL = BL * NT1         # 128 score columns, c = b*32 + j*8 + t5


def build_nc():
    nc = bacc.Bacc(None, target_bir_lowering=False)
    pos_e = nc.declare_dram_parameter("position", [BL, D], F32, isOutput=False)
    opt_e = nc.declare_dram_parameter("options_bf", [BL, S, D], BF16,
                                      isOutput=False)
    optT_e = nc.declare_dram_parameter("optionsT_bf", [BL, D, S], BF16,
                                       isOutput=False)
    w1_e = nc.declare_dram_parameter("W1", [D, U], F32, isOutput=False)
    b1_e = nc.declare_dram_parameter("b1", [U], F32, isOutput=False)
    w2_e = nc.declare_dram_parameter("W2", [D, U], F32, isOutput=False)
    b2_e = nc.declare_dram_parameter("b2", [U], F32, isOutput=False)
    v_e = nc.declare_dram_parameter("V", [U, 1], F32, isOutput=False)
    ctx_e = nc.declare_dram_parameter("context", [BL, D], F32, isOutput=True)
    wts_e = nc.declare_dram_parameter("weights", [BL, S, 1], F32, isOutput=True)

    with tile.TileContext(nc) as tc, ExitStack() as ctx:
        const = ctx.enter_context(tc.tile_pool(name="const", bufs=1))
        optT_pool = ctx.enter_context(tc.tile_pool(name="optT", bufs=8))
        tanh_pool = ctx.enter_context(tc.tile_pool(name="tanh", bufs=8))
        srow_sb_pool = ctx.enter_context(tc.tile_pool(name="srowsb", bufs=4))

        # ---- constants -----------------------------------------------------
        ones = const.tile([128, 128], F32)
        nc.vector.memset(ones[:], 1.0)
        ident = const.tile([128, 128], F32)
        nc.gpsimd.affine_select(
            ident[:], ones[:], pattern=[[1, 128]],
            compare_op=mybir.AluOpType.is_equal, fill=0.0,
            base=0, channel_multiplier=-1,
        )
        # bm128T[b, c] = 1 when score column c belongs to batch b (c//32 == b)
        bm_lo = const.tile([BL, NCOL], F32)
        nc.gpsimd.affine_select(
            bm_lo[:], ones[0:BL, :], pattern=[[1, NCOL]],
            compare_op=mybir.AluOpType.is_ge, fill=0.0,
            base=0, channel_multiplier=-NT1,
        )
        bm_hi = const.tile([BL, NCOL], F32)
        nc.gpsimd.affine_select(
            bm_hi[:], ones[0:BL, :], pattern=[[1, NCOL]],
            compare_op=mybir.AluOpType.is_ge, fill=0.0,
            base=-NT1, channel_multiplier=-NT1,
        )
        bm128T = const.tile([BL, NCOL], F32)
        nc.vector.tensor_tensor(bm128T[:], bm_lo[:], bm_hi[:],
                                mybir.AluOpType.subtract)

        # ---- weights -------------------------------------------------------
        w2f = const.tile([128, 2, U], F32)
        nc.scalar.dma_start(w2f[:], w2_e[:].rearrange("(c p) u -> p c u", p=128))
        w2b = const.tile([128, 2, U], BF16)
        nc.vector.tensor_copy(w2b[:], w2f[:])
        w1f = const.tile([128, 2, U], F32)
        nc.scalar.dma_start(w1f[:], w1_e[:].rearrange("(c p) u -> p c u", p=128))
        vf = const.tile([128, 2, 1], F32)
        nc.scalar.dma_start(vf[:], v_e[:].rearrange("(c p) o -> p c o", p=128))
        vb = const.tile([128, 2, 1], BF16)
        nc.vector.tensor_copy(vb[:], vf[:])

        b1f = const.tile([1, U], F32)
        nc.scalar.dma_start(b1f[:], b1_e[:].rearrange("(o u) -> o u", o=1))
        b2f = const.tile([1, U], F32)
        nc.scalar.dma_start(b2f[:], b2_e[:].rearrange("(o u) -> o u", o=1))
        b12 = const.tile([1, U], F32)
        nc.vector.tensor_tensor(b12[:], b1f[:], b2f[:], mybir.AluOpType.add)

        posf = const.tile([BL, D], F32)
        nc.scalar.dma_start(posf[:], pos_e[:])

        # ---- options: natural bf16 resident (for pass 2) --------------------
        optb = {}
        for b in range(BL):
            ob = const.tile([128, NT5, 4, D], BF16, tag=f"optb_{b}")
            nc.gpsimd.dma_start(
                ob[:],
                opt_e[b].rearrange("(f t p) d -> p f t d", p=128, t=4))
            optb[b] = ob

        # packed score rows [r=(b,t5), s_local]
        packed = const.tile([BL * NT5, 512], F32)
        # scoreT columns [s_lo, (b, j, t5)]
        scoreT = const.tile([128, BL, 4, NT5], F32)

        def emit_prologue():
            # transpose b1+b2 and position into U-on-partitions layout
            with ExitStack() as prol_scope:
                prol = prol_scope.enter_context(
                    tc.tile_pool(name="prol", bufs=1, space="PSUM"))
                emit_prologue.out = _prologue_body(prol)

        def _prologue_body(prol):
            b12T = const.tile([128, 2], F32)
            posT = const.tile([128, 2, BL], F32)
            for c in range(2):
                tp = prol.tile([128, 128], F32, tag="smalltr")
                nc.tensor.transpose(tp[0:128, 0:1], b12[:, c * 128:(c + 1) * 128],
                                    ident[0:1, 0:1])
                nc.vector.tensor_copy(b12T[:, c:c + 1], tp[0:128, 0:1])
                tp2 = prol.tile([128, 128], F32, tag="smalltr")
                nc.tensor.transpose(tp2[0:128, 0:BL],
                                    posf[:, c * 128:(c + 1) * 128],
                                    ident[0:BL, 0:BL])
                nc.vector.tensor_copy(posT[:, c, :], tp2[0:128, 0:BL])
            posbias = const.tile([128, 2, BL], F32)
            for uc in range(2):
                pb = prol.tile([128, 128], F32, tag="smalltr")
                for dc in range(2):
                    nc.tensor.matmul(pb[0:128, 0:BL],
                                     w1f[:, dc, uc * 128:(uc + 1) * 128],
                                     posT[:, dc, :],
                                     start=(dc == 0), stop=(dc == 1))
                nc.scalar.activation(posbias[:, uc, :], pb[0:128, 0:BL],
                                     AF.Identity, bias=b12T[:, uc:uc + 1])
            return posbias

        with ExitStack() as p1:
            proj_pool = p1.enter_context(
                tc.tile_pool(name="proj", bufs=3, space="PSUM"))
            srow_pool = p1.enter_context(
                tc.tile_pool(name="srow", bufs=4, space="PSUM"))

            # ---- pass 1: scores -------------------------------------------
            posbias = None
            for b in range(BL):
                for t5 in range(NT5):
                    r = b * NT5 + t5
                    oT = optT_pool.tile([128, 2, 512], BF16)
                    nc.sync.dma_start(
                        oT[:],
                        optT_e[b, :, t5 * 512:(t5 + 1) * 512].rearrange(
                            "(c p) s -> p c s", p=128),
                    )
                    srow = srow_pool.tile([1, 512], F32)
                    pjs = []
                    for uc in range(2):
                        pj = proj_pool.tile([128, 512], F32)
                        for dc in range(2):
                            nc.tensor.matmul(
                                pj[:], w2b[:, dc, uc * 128:(uc + 1) * 128],
                                oT[:, dc, :],
                                start=(dc == 0), stop=(dc == 1))
                        pjs.append(pj)
                    if posbias is None:
                        # emitted after tile 0's proj matmuls so the PE
                        # stream starts on work that needs only w2b + oT[0]
                        emit_prologue()
                        posbias = emit_prologue.out
                    for uc in range(2):
                        th = tanh_pool.tile([128, 512], BF16)
                        nc.scalar.activation(th[:], pjs[uc][:], AF.Tanh,
                                             bias=posbias[:, uc, b:b + 1])
                        nc.tensor.matmul(srow[:], vb[:, uc, :], th[:],
                                         start=(uc == 0), stop=(uc == 1))
                    srow_sb = srow_sb_pool.tile([1, 512], F32)
                    nc.vector.tensor_copy(srow_sb[:], srow[:])
                    # gather this row across partitions via a tiny DMA
                    nc.gpsimd.dma_start(packed[r:r + 1, :], srow_sb[:])

        # ---- softmax + pass 2 ---------------------------------------------
        with ExitStack() as p2:
            strx = p2.enter_context(tc.tile_pool(name="strx", bufs=2,
                                                 space="PSUM"))
            eps = p2.enter_context(tc.tile_pool(name="eps", bufs=1,
                                                space="PSUM"))

            # transpose packed rows into scoreT columns (c = b*32 + j*8 + t5)
            for j in range(4):
                tp = strx.tile([128, 32], F32)
                nc.tensor.transpose(tp[:], packed[:, j * 128:(j + 1) * 128],
                                    ident[0:BL * NT5, 0:BL * NT5])
                nc.vector.tensor_copy(
                    scoreT[:, :, j, :],
                    tp[:].rearrange("p (b t) -> p b t", b=BL))

            expT = const.tile([128, NCOL], F32)
            score_flat = scoreT[:].rearrange("p b j t -> p (b j t)")
            nc.scalar.activation(expT[:], score_flat, AF.Exp)

            colsum_ps = eps.tile([1, NCOL], F32, tag="colsum")
            nc.tensor.matmul(colsum_ps[:], ones[:, 0:1], expT[:])
            colsum = const.tile([1, NCOL], F32)
            nc.vector.tensor_copy(colsum[:], colsum_ps[:])
            zrow = const.tile([1, BL], F32)
            nc.vector.tensor_reduce(
                zrow[:], colsum[:].rearrange("o (b t) -> o b t", b=BL),
                mybir.AxisListType.X, mybir.AluOpType.add)
            invzr = const.tile([1, BL], F32)
            nc.vector.reciprocal(invzr[:], zrow[:])
            invz_ps = eps.tile([BL, 1], F32, tag="invz")
            nc.tensor.transpose(invz_ps[:], invzr[:], ident[0:1, 0:1])
            invz4 = const.tile([BL, 1], F32)
            nc.vector.tensor_copy(invz4[:], invz_ps[:])

            # invZmat[p, c] = 1/Z_{batch(c)}  via ones^T @ (bm128T * invz4)
            bmsc = const.tile([BL, NCOL], F32)
            nc.vector.tensor_scalar(bmsc[:], bm128T[:], invz4[:], None,
                                    mybir.AluOpType.mult)
            invzmat_ps = eps.tile([128, NCOL], F32, tag="invzmat")
            nc.tensor.matmul(invzmat_ps[:], ones[0:BL, :], bmsc[:])

            # normalized weights, transposed layout [s_lo, c]
            wT = const.tile([128, NCOL], F32)
            nc.vector.tensor_tensor(wT[:], expT[:], invzmat_ps[:],
                                    mybir.AluOpType.mult)
            wTb = const.tile([128, NCOL], BF16)
            nc.vector.tensor_copy(wTb[:], wT[:])

            # back to row layout [(b,j,t5), s_lo] for a contiguous DMA out
            wrows_ps = eps.tile([128, 128], F32, tag="wrows")
            nc.tensor.transpose(wrows_ps[:], wT[:], ident[:])
            wrows = const.tile([128, 128], F32)
            nc.vector.tensor_copy(wrows[:], wrows_ps[:])
            for b in range(BL):
                wv = wts_e[b].rearrange("(t j p) o -> j t (p o)", p=128, j=4)
                for j in range(4):
                    nc.gpsimd.dma_start(
                        wv[j],
                        wrows[b * NT1 + j * NT5:b * NT1 + (j + 1) * NT5, :])

            # pass 2: context[b] = sum_s w[b,s] * options[b,s,:]  (bf16)
            ctx_ps = eps.tile([1, BL * D], F32, tag="ctxps")
            for b in range(BL):
                k = 0
                for t5 in range(NT5):
                    for j in range(4):
                        c = b * NT1 + j * NT5 + t5
                        nc.tensor.matmul(
                            ctx_ps[0:1, b * D:(b + 1) * D],
                            wTb[:, c:c + 1],
                            optb[b][:, t5, j, :],
                            start=(k == 0), stop=(k == NT1 - 1))
                        k += 1
            ctxrow = const.tile([1, BL * D], F32)
            nc.vector.tensor_copy(ctxrow[:], ctx_ps[:])
            nc.sync.dma_start(
                ctx_e[:].rearrange("(o b) d -> o (b d)", o=1), ctxrow[:])
    nc.finalize()
    return nc


_NC = None


def _get_nc():
    global _NC
    if _NC is None:
        _NC = build_nc()
    return _NC


def make_in_maps(position, options, W1, b1, W2, b2, V, bv=None):
    import ml_dtypes
    position = np.ascontiguousarray(position, dtype=np.float32)
    options = np.asarray(options, dtype=np.float32)
    opt_bf = np.ascontiguousarray(options.astype(ml_dtypes.bfloat16))
    optT_bf = np.ascontiguousarray(opt_bf.transpose(0, 2, 1))
    W1 = np.ascontiguousarray(W1, dtype=np.float32)
    b1 = np.ascontiguousarray(b1, dtype=np.float32)
    W2 = np.ascontiguousarray(W2, dtype=np.float32)
    b2 = np.ascontiguousarray(b2, dtype=np.float32)
    V = np.ascontiguousarray(V, dtype=np.float32)
    in_maps = []
    for c in range(NCORES):
        bs = slice(c * BL, (c + 1) * BL)
        in_maps.append({
            "position": position[bs], "options_bf": opt_bf[bs],
            "optionsT_bf": optT_bf[bs],
            "W1": W1, "b1": b1, "W2": W2, "b2": b2, "V": V,
        })
    return in_maps


def run_spmd(in_maps, trace=False):
    from concourse.bass_utils import run_bass_kernel_spmd
    return run_bass_kernel_spmd(_get_nc(), in_maps, list(range(NCORES)),
                                trace=trace)


def kernel(position, options, W1, b1, W2, b2, V, bv):
    res = run_spmd(make_in_maps(position, options, W1, b1, W2, b2, V)).results
    context = np.concatenate([res[c]["context"] for c in range(NCORES)], axis=0)
    weights = np.concatenate([res[c]["weights"] for c in range(NCORES)], axis=0)
    return context, weights
